# revision 9
# baseline (speedup 1.0000x reference)
"""Trainium2 Bass kernel for nn_CustomConv1D (nealmon-softmax windowed conv).

Computation (reference):
    w = softmax(param5 * i + param6 * i^2),  i = 1..64          # (64,)
    out[b, t, c] = sum_{k<64, ci<10} x[b, 64*t + k, ci] * w[k]  # (256, 512, 10)

Key observation: x[b] flattened row-major is (32768*10,) f32, and window t of
batch b occupies 640 *consecutive* elements [t*640, (t+1)*640).  So the whole
job is: for every contiguous 640-element chunk, compute a weighted sum
(weights = w repeated 10x, since the channel dim is innermost), then broadcast
that scalar to 10 output channels.

Strategy (pure data-parallel over batch, 8 cores x 32 batches):
  - Per core: 32*32768*10 = 10,485,760 contiguous f32 (40 MiB), processed as
    18 slabs of (128 partitions x F elems), F mostly 5120 (2.62 MB) with a
    shrinking tail [2560, 3840, 2560, 1280] that cuts the pipeline drain.
    Each partition holds whole windows -> none straddles a partition boundary.
  - Per slab on-device:
      1. DVE tensor_reduce  (128, 512, 10) -X-> (128, 512)     # channel sum
      2. DVE tensor_mul     (128, 512) * W_tile                # per-lag weight
      3. DVE tensor_reduce  (128, 8, 64) -X-> (128, 8)         # lag sum
      4. ACT copy broadcast (128, 8) -> (128, 8, 10)           # out channels
      5. DMA out (on the ACT HWDGE queue, separate from loads)
  - Weights are computed on host (64-elem softmax) and shipped pre-tiled as a
    (128, 512) constant so no broadcast AP is needed on the multiply.

Measured (slope over For_i-looped NEFFs, which cancels axon RPC overhead):
  DMA-only floor ~124 us (41.9 MB @ ~338 GB/s, the per-NC HBM wall); DVE busy
  ~106 us (TimelineSim), hidden under the DMA shadow.  With the plain For_i
  back-edge (all-engine barrier + drain) the slope includes the full
  startup+drain per rep (~132-157 us depending on neighbor contention);
  For_i(staggered_reset=True) + x2 inner unroll lets consecutive reps
  pipeline, so the slope measures steady-state throughput: ~127 us/rep
  (loads + the 0.65 MB of output stores + ~1-2 us stage sync).  Output
  stores ride the ACT HWDGE ring, not the SP ring the loads use — keeping
  store descriptors out of the load FIFO measured ~3 us/rep faster.
"""

import numpy as np

import concourse.bass as bass
import concourse.bacc as bacc
import concourse.mybir as mybir
import concourse.tile as tile
from concourse.bass_utils import run_bass_kernel_spmd

# Problem shape (hardcoded per contract: kernel.py must be self-contained).
B, T, C = 256, 32768, 10
KW = 64
N_CORES = 8
B_PER_CORE = B // N_CORES                      # 32
NWIN = T // KW                                 # 512 windows per batch
ELEMS_PER_CORE = B_PER_CORE * T * C            # 10,485,760
# Per-partition slab sizes (each a multiple of 640 so windows never straddle
# partitions; sum = 81920 = ELEMS_PER_CORE/128).  The shrinking tail slabs cut
# the pipeline drain: after the last big load lands, only a small final DVE
# chain + store remain (HW-measured ~9 us faster than uniform 16x5120).
SIZES = [5120] * 14 + [2560] + [3840, 2560, 1280]
WIN_PER_PART = max(SIZES) // (KW * C)          # 8 windows (max, for W tile)
OUT_ELEMS_PER_CORE = B_PER_CORE * NWIN * C     # 163,840
XBUFS, RBUFS = 6, 3

_FP32 = mybir.dt.float32

_cache = {}


def _build_bass(reps: int = 1, unroll: bool = False, staggered: bool = False,
                inner_unroll: int = 1):
    """Build the single-core Bass program (same NEFF runs SPMD on all cores).

    reps > 1 wraps the pipeline in a tc.For_i loop repeating it on the same
    data — used only for slope-based HW timing in test.py/bench.py.
    unroll=True repeats the body as straight-line code instead (for
    TimelineSim, which can't resolve For_i reg branches without an executor).
    staggered=True uses For_i(staggered_reset=True): no all-engine barrier at
    the back edge, so consecutive reps pipeline (loads of rep k+1 overlap the
    compute drain of rep k) — the slope then measures steady-state throughput.
    inner_unroll repeats the body that many times per For_i iteration,
    halving/quartering the per-rep staggered-reset stage overhead.
    """
    nc = bacc.Bacc("TRN2", target_bir_lowering=False, debug=False,
                   num_devices=N_CORES)

    x_d = nc.dram_tensor("x", (ELEMS_PER_CORE,), _FP32, kind="ExternalInput").ap()
    w_d = nc.dram_tensor("w", (128, WIN_PER_PART * KW), _FP32,
                         kind="ExternalInput").ap()  # (128, 512) = tile(w, 8)
    out_d = nc.dram_tensor("out", (OUT_ELEMS_PER_CORE,), _FP32,
                           kind="ExternalOutput").ap()

    with tile.TileContext(nc) as tc:
        with (
            tc.tile_pool(name="const", bufs=1) as cpool,
            tc.tile_pool(name="x", bufs=XBUFS) as xpool,
            tc.tile_pool(name="r1", bufs=RBUFS) as r1pool,
            tc.tile_pool(name="r2", bufs=RBUFS) as r2pool,
            tc.tile_pool(name="s", bufs=RBUFS) as spool,
            tc.tile_pool(name="o", bufs=RBUFS) as opool,
            tc.tile_pool(name="ob", bufs=2) as obpool,
        ):
            wt = cpool.tile([128, WIN_PER_PART * KW], _FP32)
            nc.scalar.dma_start(wt[:], w_d)

            def body():
                # Uniform-region outputs accumulate in one SBUF block and
                # store ONCE (after slab 13), so 14 small stores stop
                # interleaving into the load stream (sim -2.7 us; HW >= par).
                ob = obpool.tile([128, 14 * 80], _FP32, tag="ob")
                base = 0
                obase = 0
                for idx, f in enumerate(SIZES):
                    wpp = f // (KW * C)
                    of = wpp * C
                    xt = xpool.tile([128, f], _FP32, tag="x")
                    nc.sync.dma_start(
                        xt[:],
                        x_d[base:base + 128 * f].rearrange("(p f) -> p f", f=f))

                    # 1. channel sum: (128, f/10, 10) -> (128, f/10)
                    r1 = r1pool.tile([128, f // C], _FP32, tag="r1")
                    nc.vector.reduce_sum(
                        r1[:], xt[:].rearrange("p (g c) -> p g c", c=C),
                        axis=mybir.AxisListType.X)

                    # 2. per-lag weights (wt is tile(w, 8); prefix works for
                    #    smaller slabs since the pattern is 64-periodic)
                    r2 = r2pool.tile([128, f // C], _FP32, tag="r2")
                    nc.vector.tensor_mul(r2[:], r1[:], wt[:, :f // C])

                    # 3. lag sum: (128, wpp, 64) -> (128, wpp)
                    st = spool.tile([128, wpp], _FP32, tag="s")
                    nc.vector.reduce_sum(
                        st[:], r2[:].rearrange("p (t k) -> p t k", k=KW),
                        axis=mybir.AxisListType.X)

                    # 4. broadcast to 10 channels (ACT — off the DVE hot path)
                    if idx < 14:
                        nc.scalar.copy(
                            ob[:, idx * 80:(idx + 1) * 80].rearrange(
                                "p (t c) -> p t c", c=C),
                            st[:].unsqueeze(2).broadcast_to([128, wpp, C]))
                        if idx == 13:
                            # stores ride the ACT HWDGE ring so they never
                            # serialize inside the SP ring's load FIFO
                            # (HW A/B: -3 us/rep vs sync-ring stores)
                            nc.scalar.dma_start(
                                out_d[0:14 * 128 * 80].rearrange(
                                    "(i p j) -> p i j", i=14, p=128, j=80),
                                ob[:].rearrange("p (i j) -> p i j", j=80))
                    else:
                        ot = opool.tile([128, of], _FP32, tag="o")
                        nc.scalar.copy(
                            ot[:].rearrange("p (t c) -> p t c", c=C),
                            st[:].unsqueeze(2).broadcast_to([128, wpp, C]))
                        nc.scalar.dma_start(
                            out_d[obase:obase + 128 * of].rearrange(
                                "(p f) -> p f", f=of),
                            ot[:])
                    base += 128 * f
                    obase += 128 * of

            if reps > 1 and unroll:
                for _ in range(reps):
                    body()
            elif reps > 1:
                with tc.For_i(0, reps // inner_unroll, 1,
                              staggered_reset=staggered):
                    for _ in range(inner_unroll):
                        body()
            else:
                body()

    nc.compile()
    return nc


def _weights(param5: np.ndarray, param6: np.ndarray) -> np.ndarray:
    i = np.arange(1, KW + 1, dtype=np.float32)
    ll = np.float32(param5) * i + np.float32(param6) * i * i
    ll = ll - ll.max()
    e = np.exp(ll)
    w = (e / e.sum()).astype(np.float32)
    return np.tile(w, (128, WIN_PER_PART)).copy()  # (128, 512)


def kernel(x: np.ndarray, param5: np.ndarray, param6: np.ndarray):
    x = np.ascontiguousarray(x, dtype=np.float32)
    assert x.shape == (B, T, C)

    if "nc" not in _cache:
        _cache["nc"] = _build_bass()
    nc = _cache["nc"]

    w_tiled = _weights(param5, param6)
    shards = x.reshape(N_CORES, ELEMS_PER_CORE)
    in_maps = [{"x": shards[c], "w": w_tiled} for c in range(N_CORES)]

    res = run_bass_kernel_spmd(nc, in_maps, core_ids=list(range(N_CORES)))
    _cache["last_results"] = res

    out = np.empty((B, NWIN, C), dtype=np.float32)
    for c in range(N_CORES):
        out[c * B_PER_CORE:(c + 1) * B_PER_CORE] = (
            res.results[c]["out"].reshape(B_PER_CORE, NWIN, C))
    return out

